# revision 1
# baseline (speedup 1.0000x reference)
"""CrossTransformerLayer on 8 TRN2 NeuronCores.

Sharding: core c -> (batch b = c//2, q-half = c%2). Each core computes its
512 query rows of its batch end-to-end (k/v over the full 1024-token x1
sequence), so no cross-core collectives are needed. The host slices inputs,
pre-transposes the attention bias to [head, k, q], and reassembles the 8
output slices.

Precision plan (rel-err budget 2e-2; measured ~3e-3):
  - QKV/O projection weights and the attention p@v path run in fp8e4m3
    with DoubleRow perf mode (stationary+moving both fp8, contraction 256
    per matmul). Weights are host-prescaled x64 (descaled in the PSUM->SBUF
    copy); v is carried x4, exp(p) is carried /16, oT is carried x4, with
    the scales cancelling exactly through the rowsum trick.
  - Scores (q.k^T, contraction 64) and the FFN stay bf16: fp8 in the FFN
    measurably busts the error budget, and the score matmul cannot pack a
    128-row contraction.
  - Layernorm stats/softmax logits/residuals/output stay fp32.

Engine balance: exp() dominates attention (ACT engine), so the attention
bias lands in PSUM via identity-matmul preloads (PE), keeping DVE nearly
idle there; layernorm application and half of the transpose/projection
PSUM->SBUF copies run on ACT (Identity with per-partition scale/bias),
the other half on DVE.
"""

import math
import sys

sys.path.insert(0, "/opt/trn_rl_repo")

from contextlib import ExitStack

import numpy as np

import concourse.bass as bass
import concourse.tile as tile
from concourse import bacc, mybir
from concourse.masks import make_identity

F32 = mybir.dt.float32
BF16 = mybir.dt.bfloat16
E4 = mybir.dt.float8e4
DR = mybir.MatmulPerfMode.DoubleRow

B = 4
S = 1024   # full (k) sequence
Sq = 512   # query rows per core
H = 1024
NH = 16
Dh = 64    # head dim
FF = 4096
P = 128
NKT = S // P    # 8 k-token tiles
NFC = H // P    # 8 feature chunks
NTC = Sq // P   # 4 q-token tiles
NFFC = FF // P  # 32 ff chunks
EPS = 1e-5
SCALE = float(Dh) ** -0.5
SW = 64.0   # weight prescale (wq/wk/wv/wo stored as 64*w in fp8)
SV = 4.0    # v carried as 4*v in v_aug
SE = 16.0   # exp carried as e/16
SO = SV  # oT ends up carried x4 via the rowsum cancellation
AF = mybir.ActivationFunctionType
OP = mybir.AluOpType


def _pbcast(ap: bass.AP, parts: int) -> bass.AP:
    """[.., N] access pattern -> [parts, .., N] with partition step 0."""
    return bass.AP(
        tensor=ap.tensor,
        offset=ap.offset,
        ap=[[0, parts]] + [list(d) for d in ap.ap],
    )


def build_program(ln_affine=True, with_biases=True):
    nc = bacc.Bacc("TRN2", target_bir_lowering=False, debug=False)

    x1_d = nc.dram_tensor("x1", (S, H), BF16, kind="ExternalInput")
    x2h_d = nc.dram_tensor("x2h", (Sq, H), F32, kind="ExternalInput")
    biasT_d = nc.dram_tensor("biasT", (NH, S, Sq), E4, kind="ExternalInput")
    wq_d = nc.dram_tensor("wq", (H, H), E4, kind="ExternalInput")
    wk_d = nc.dram_tensor("wk", (H, H), E4, kind="ExternalInput")
    wv_d = nc.dram_tensor("wv", (H, H), E4, kind="ExternalInput")
    wo_d = nc.dram_tensor("wo", (H, H), E4, kind="ExternalInput")
    bq_d = nc.dram_tensor("bq_pc", (P, NFC), F32, kind="ExternalInput")
    bk_d = nc.dram_tensor("bk_pc", (P, NFC), F32, kind="ExternalInput")
    bv_d = nc.dram_tensor("bv4", (H,), F32, kind="ExternalInput")
    bo_d = nc.dram_tensor("bo", (H,), F32, kind="ExternalInput")
    # fp8 value+residual weight streams, host-packed into the exact SBUF
    # layout (one large-contiguous DMA per chunk): [g, p, kc, set, cols]
    w1_d = nc.dram_tensor(
        "w1p", (NFFC // 2, P, NFC, 2, 256), E4, kind="ExternalInput"
    )
    b1_d = nc.dram_tensor("b1_pc", (P, NFFC), F32, kind="ExternalInput")
    w2_d = nc.dram_tensor(
        "w2p", (NFFC // 4, P, 4, 2, H), E4, kind="ExternalInput"
    )
    b2_d = nc.dram_tensor("b2", (H,), F32, kind="ExternalInput")
    ln1g_d = nc.dram_tensor("ln1_g", (H,), F32, kind="ExternalInput")
    ln1b_d = nc.dram_tensor("ln1_b", (H,), F32, kind="ExternalInput")
    ln2g_d = nc.dram_tensor("ln2_g", (H,), F32, kind="ExternalInput")
    ln2b_d = nc.dram_tensor("ln2_b", (H,), F32, kind="ExternalInput")
    lnfg_d = nc.dram_tensor("lnf_g", (H,), F32, kind="ExternalInput")
    lnfb_d = nc.dram_tensor("lnf_b", (H,), F32, kind="ExternalInput")
    out_d = nc.dram_tensor("out", (Sq, H), F32, kind="ExternalOutput")

    def _mmdr(out, lhsT, rhs, start, stop):
        nc.tensor.matmul(out, lhsT, rhs, start=start, stop=stop, perf_mode=DR)

    def _layer_norm(pool, y_out, x_in, g_b, b_b, eps_t, apply_on="act"):
        """y = (x - mean)/sqrt(var+eps) [* g + b] on a [128, H] tile.

        Stats on DVE; apply on ACT (Identity with per-partition scale/bias)
        or on GPSIMD (tensor_scalar) when ACT is the busier engine.
        """
        stats = pool.tile([P, 2, 6], F32, tag="ln_stats", name="stats")
        nc.vector.bn_stats(stats[:, 0, :], x_in[:, 0:512])
        nc.vector.bn_stats(stats[:, 1, :], x_in[:, 512:1024])
        mv = pool.tile([P, 2], F32, tag="ln_mv", name="mv")
        nc.vector.bn_aggr(mv, stats)
        # rstd = exp(-0.5*ln(var+eps)): Ln and Exp share one ACT table set
        # with the attention softmax, avoiding Sqrt-set reloads
        lnv = pool.tile([P, 1], F32, tag="ln_lnv", name="lnv")
        nc.scalar.activation(lnv, mv[:, 1:2], AF.Ln, bias=eps_t, scale=1.0)
        rstd = pool.tile([P, 1], F32, tag="ln_rstd", name="rstd")
        nc.scalar.activation(rstd, lnv, AF.Exp, bias=0.0, scale=-0.5)
        if apply_on == "pool":
            # direct (x - m) * r on GPSIMD — no negmr precompute needed
            nc.gpsimd.tensor_scalar(
                y_out, x_in, mv[:, 0:1], rstd[:, 0:1],
                op0=OP.subtract, op1=OP.mult,
            )
        else:
            negmr = pool.tile([P, 1], F32, tag="ln_negmr", name="negmr")
            nc.vector.tensor_scalar(
                negmr, mv[:, 0:1], rstd[:, 0:1], -1.0, op0=OP.mult, op1=OP.mult
            )
            nc.scalar.activation(
                y_out, x_in, AF.Identity, bias=negmr[:, 0:1], scale=rstd[:, 0:1]
            )
        if ln_affine:
            nc.vector.tensor_mul(y_out, y_out, g_b)
            nc.vector.tensor_add(y_out, y_out, b_b)

    with tile.TileContext(nc) as tc, ExitStack() as top:
        persist = top.enter_context(tc.tile_pool(name="persist", bufs=1))
        ident = persist.tile([P, P], BF16, tag="ident")
        make_identity(nc, ident)
        # fp32 identity scaled by SO*SW: preloads 256*x2 into the o-proj PSUM
        # so the residual add rides the matmul accumulation instead of DVE
        id256 = persist.tile([P, P], F32, tag="id256")
        nc.vector.tensor_scalar(id256, ident, float(SO * SW), None, op0=OP.mult)
        oT = persist.tile([P, NFC, Sq], E4, tag="oT")       # 4*o_norm [p,fc,q]

        xp = top.enter_context(tc.tile_pool(name="xp", bufs=1))
        x_sb = xp.tile([P, NTC, H], F32, tag="x")   # [p, tc, f], ph 4-7
        x2_sb = xp.tile([P, NTC, H], F32, tag="x2")  # resident x2h, ph 1-4
        w4pool = top.enter_context(tc.tile_pool(name="ph4w", bufs=1))
        with (
            tc.tile_pool(name="qkv", bufs=1) as qkvp,           # phases 1-3
            tc.tile_pool(name="bias_s", bufs=2) as bpool,
            tc.tile_pool(name="expp", bufs=4) as epool,
            tc.tile_pool(name="rin", bufs=2) as rpool,
        ):
            qT = qkvp.tile([P, NFC, Sq], BF16, tag="qT")        # [p, fc, q]
            kT = qkvp.tile([P, NFC, S], BF16, tag="kT")         # [p, fc, k]
            v_aug = qkvp.tile([P, NKT, NH * 65], E4, tag="vaug")

            # ------------ Phase 1+2: LN, transpose, QKV projections ---------
            # y1T lives through attention: the k-projection is interleaved
            # into the head loop so softmax exp (the attention bottleneck)
            # can start as soon as the first kT chunk lands.
            y1T = qkvp.tile([P, NFC, S], E4, tag="y1T")
            y2T = qkvp.tile([P, NFC, Sq], E4, tag="y2T")
            wk_sb = qkvp.tile([P, NFC, H], E4, tag="wk")
            if True:
                with (
                    tc.tile_pool(name="ph1", bufs=4) as ph1,
                    tc.tile_pool(name="ph1x", bufs=NKT) as ph1x,
                    tc.tile_pool(name="trp", bufs=4) as trp,
                    tc.tile_pool(name="ph1w", bufs=4) as ph1w,
                    tc.tile_pool(name="ph1c", bufs=1) as ph1c,
                    tc.tile_pool(name="wload", bufs=1) as wpool,
                    tc.tile_pool(name="vecs", bufs=1) as vecs,
                    tc.tile_pool(
                        name="ph2ps", bufs=3, space=bass.MemorySpace.PSUM
                    ) as ps2,
                ):
                    eps_t = ph1c.tile([P, 1], F32, tag="eps")
                    nc.vector.memset(eps_t, EPS)
                    ln1g_b = ln1b_b = ln2g_b = ln2b_b = None
                    if ln_affine:
                        ln1g_b = ph1c.tile([P, H], F32, tag="ln1g")
                        ln1b_b = ph1c.tile([P, H], F32, tag="ln1b")
                        ln2g_b = ph1c.tile([P, H], F32, tag="ln2g")
                        ln2b_b = ph1c.tile([P, H], F32, tag="ln2b")
                        nc.gpsimd.dma_start(ln1g_b, _pbcast(ln1g_d[:], P))
                        nc.gpsimd.dma_start(ln1b_b, _pbcast(ln1b_d[:], P))
                        nc.gpsimd.dma_start(ln2g_b, _pbcast(ln2g_d[:], P))
                        nc.gpsimd.dma_start(ln2b_b, _pbcast(ln2b_d[:], P))

                    def _w_full(wd, name, tag):
                        w_sb = wpool.tile([P, NFC, H], E4, tag=tag, name=name)
                        nc.gpsimd.dma_start(
                            w_sb, wd.rearrange("(kc p) f -> p kc f", p=P)
                        )
                        return w_sb

                    bq_sb = bv_b = None
                    if with_biases:
                        bq_sb = vecs.tile([P, NFC], F32, tag="bq")
                        bk_sb = qkvp.tile([P, NFC], F32, tag="bk")
                        bv_b = vecs.tile([P, H], F32, tag="bvb")
                        nc.gpsimd.dma_start(bq_sb, bq_d[:, :])
                        nc.gpsimd.dma_start(bk_sb, bk_d[:, :])
                        nc.gpsimd.dma_start(bv_b, _pbcast(bv_d[:], P))

                    # ones columns of v_aug (slot 64 of each head) = 1.0
                    ones_view = v_aug[:, :, :].rearrange(
                        "p t (h j) -> p t h j", j=65
                    )[:, :, :, 64:65]
                    nc.vector.memset(ones_view, 1.0)

                    wv_sb = _w_full(wv_d[:, :], "wv_sb", tag="wv")

                    xp_i = [0]

                    def _xpose_block(yt, dst_slice, on_act):
                        """DMA-xbar transpose of a [P, H] bf16 tile into a
                        bf16 staging tile (runs on the DMA engines), then one
                        quantizing copy to the fp8 destination. Keeps PE, DVE
                        and ACT free of transpose traffic."""
                        ytT = trp.tile([P, NFC, P], BF16, tag="ytT", name="ytT")
                        nc.sync.dma_start_transpose(ytT, yt)
                        i = xp_i[0]
                        xp_i[0] += 1
                        if i % 3 == 2:
                            nc.scalar.copy(dst_slice, ytT[:, :, :])
                        elif i % 3 == 1:
                            nc.vector.tensor_copy(dst_slice, ytT[:, :, :])
                        else:
                            nc.gpsimd.tensor_copy(dst_slice, ytT[:, :, :])

                    for t in range(NTC):  # x2h -> y2 -> y2T (x2h kept resident)
                        eng = nc.sync if t % 2 == 0 else nc.scalar
                        eng.dma_start(
                            x2_sb[:, t, :], x2h_d[t * P:(t + 1) * P, :]
                        )
                        yt = ph1w.tile([P, H], BF16, tag="yt", name="yt")
                        _layer_norm(ph1, yt, x2_sb[:, t, :], ln2g_b, ln2b_b, eps_t)
                        _xpose_block(
                            yt, y2T[:, :, t * P:(t + 1) * P], on_act=(t % 2 == 1)
                        )

                    # pre-issue all x1 loads, split over the SP and Pool DMA
                    # queues (per-queue issue is serialized in-order)
                    x1t = []
                    for t in range(NKT):
                        xt = ph1x.tile([P, H], BF16, tag="x1t", name="xt")
                        eng = nc.sync if t % 2 == 0 else nc.gpsimd
                        eng.dma_start(xt, x1_d[t * P:(t + 1) * P, :])
                        x1t.append(xt)

                    wq_sb = _w_full(wq_d[:, :], "wq_sb", tag="wq")
                    nc.gpsimd.dma_start(
                        wk_sb, wk_d[:, :].rearrange("(kc p) f -> p kc f", p=P)
                    )

                    # q^T[fo, :] = sum_c wq[pair c, fo].T @ y2T[pair c]  (DR)
                    for fp in range(NFC // 2):
                        ps = ps2.tile([P, 2, Sq], F32, tag="mm", name="ps")
                        for i in range(2):
                            fo = 2 * fp + i
                            for c in range(NFC // 2):
                                _mmdr(
                                    ps[:, i, :],
                                    wq_sb[:, 2 * c:2 * c + 2,
                                          fo * P:(fo + 1) * P],
                                    y2T[:, 2 * c:2 * c + 2, :],
                                    start=(c == 0), stop=(c == NFC // 2 - 1),
                                )
                        dst = qT[:, 2 * fp:2 * fp + 2, :]
                        if with_biases:
                            for i in range(2):
                                fo = 2 * fp + i
                                nc.vector.tensor_scalar(
                                    qT[:, fo, :], ps[:, i, :], 1.0 / SW,
                                    bq_sb[:, fo:fo + 1],
                                    op0=OP.mult, op1=OP.add,
                                )
                        elif fp % 2 == 0:
                            nc.vector.tensor_scalar(
                                dst, ps[:, :, :], 1.0 / SW, None, op0=OP.mult
                            )
                        else:
                            nc.scalar.mul(dst, ps[:, :, :], 1.0 / SW)

                    # x1 -> y1 -> y1T, interleaved with v[t] = y1T[t].T @ wv
                    for t in range(NKT):
                        yt = ph1w.tile([P, H], BF16, tag="yt", name="yt")
                        _layer_norm(ph1, yt, x1t[t], ln1g_b, ln1b_b, eps_t,
                                    apply_on=("pool" if t >= 2 else "act"))
                        _xpose_block(
                            yt, y1T[:, :, t * P:(t + 1) * P], on_act=(t % 2 == 1)
                        )
                        ps = ps2.tile([P, 2, 512], F32, tag="mm", name="ps")
                        for nt in range(2):
                            for c in range(NFC // 2):
                                _mmdr(
                                    ps[:, nt, :],
                                    y1T[:, 2 * c:2 * c + 2, t * P:(t + 1) * P],
                                    wv_sb[:, 2 * c:2 * c + 2,
                                          nt * 512:(nt + 1) * 512],
                                    start=(c == 0), stop=(c == NFC // 2 - 1),
                                )
                        dst = v_aug[:, t, :].rearrange(
                            "p (h j) -> p h j", j=65
                        )[:, :, 0:64]
                        src = ps.rearrange("p two (h j) -> p (two h) j", j=64)
                        if with_biases:
                            tmp = ph1w.tile(
                                [P, 2, 512], F32, tag="vtmp", name="vtmp"
                            )
                            nc.vector.tensor_scalar(
                                tmp, ps, SV / SW, None, op0=OP.mult
                            )
                            nc.vector.tensor_tensor(
                                out=dst,
                                in0=tmp.rearrange(
                                    "p two (h j) -> p (two h) j", j=64
                                ),
                                in1=bv_b[:, :].rearrange(
                                    "p (h j) -> p h j", j=64
                                ),
                                op=OP.add,
                            )
                        elif t % 2 == 0:
                            nc.vector.tensor_scalar(
                                dst, src, SV / SW, None, op0=OP.mult
                            )
                        else:
                            nc.scalar.mul(dst, src, SV / SW)

                    # k^T[fo, :] = sum_c wk[pair c, fo].T @ y1T[pair c]
                    for fo in range(NFC):
                        ps = ps2.tile([P, 2, 512], F32, tag="mm", name="ps")
                        for nt in range(2):
                            for c in range(NFC // 2):
                                _mmdr(
                                    ps[:, nt, :],
                                    wk_sb[:, 2 * c:2 * c + 2,
                                          fo * P:(fo + 1) * P],
                                    y1T[:, 2 * c:2 * c + 2,
                                        nt * 512:(nt + 1) * 512],
                                    start=(c == 0), stop=(c == NFC // 2 - 1),
                                )
                        dst = kT[:, fo, :]
                        if with_biases:
                            nc.vector.tensor_scalar(
                                dst, ps[:, :, :], 1.0 / SW, bk_sb[:, fo:fo + 1],
                                op0=OP.mult, op1=OP.add,
                            )
                        elif fo % 2 == 0:
                            nc.vector.tensor_scalar(
                                dst, ps[:, :, :], 1.0 / SW, None, op0=OP.mult
                            )
                        else:
                            nc.scalar.mul(dst, ps[:, :, :], 1.0 / SW)


            # wo prefetch (consumed in phase 4, loads during attention)
            wo_sb = w4pool.tile([P, NFC, H], E4, tag="wo")
            nc.gpsimd.dma_start(
                wo_sb, wo_d[:, :].rearrange("(kc p) f -> p kc f", p=P)
            )

            # ---------------- Phase 3: attention ----------------
            # Host pre-multiplies biasT by 1/scale (x8, fp8); identity-matmul
            # preloads (PE) land it in PSUM under the bf16 score matmul.
            # One Exp per 2-bank PSUM tile computes e/16 in fp8; o and the
            # rowsum accumulate via fp8 DoubleRow over kt pairs; the /16 and
            # v's x4 cancel in the rowsum division, leaving oT = 4*o_norm.
            with (
                tc.tile_pool(name="ph3c", bufs=1) as ph3c,
                tc.tile_pool(
                    name="sc_ps", bufs=3, space=bass.MemorySpace.PSUM
                ) as scps,
                tc.tile_pool(
                    name="o_ps", bufs=2, space=bass.MemorySpace.PSUM
                ) as ops,
            ):
                eln_t = ph3c.tile([P, 1], F32, tag="eln")
                nc.vector.memset(eln_t, -math.log(SE))
                for h in range(NH):
                    hp = (h % 2) * Dh
                    fc = h // 2
                    o_ps = ops.tile([65, Sq], F32, tag="o", name="o_ps")
                    bt = bpool.tile([P, NKT, Sq], E4, tag="bt", name="bt")
                    nc.sync.dma_start(
                        bt, biasT_d[h].rearrange("(kt p) q -> p kt q", p=P)
                    )
                    for kp in range(NKT // 2):
                        sc = scps.tile([P, 2, Sq], F32, tag="sc", name="sc_ps")
                        for j in range(2):
                            kt = 2 * kp + j
                            nc.tensor.matmul(
                                sc[:, j, :], ident, bt[:, kt, :],
                                start=True, stop=False,
                            )
                            nc.tensor.matmul(
                                sc[:, j, :],
                                kT[hp:hp + Dh, fc, kt * P:(kt + 1) * P],
                                qT[hp:hp + Dh, fc, :],
                                start=False, stop=True,
                            )
                        e2 = epool.tile([P, 2, Sq], E4, tag="expT", name="e2")
                        nc.scalar.activation(
                            e2, sc[:, :, :], AF.Exp,
                            bias=eln_t[:, 0:1], scale=SCALE,
                        )
                        _mmdr(
                            o_ps,
                            v_aug[:, 2 * kp:2 * kp + 2, h * 65:(h + 1) * 65],
                            e2[:, :, :],
                            start=(kp == 0), stop=(kp == NKT // 2 - 1),
                        )
                    rinv = rpool.tile([1, Sq], F32, tag="rinv", name="rinv")
                    nc.vector.reciprocal(rinv, o_ps[64:65, :])
                    rb = rpool.tile([Dh, Sq], F32, tag="rb", name="rb")
                    nc.gpsimd.partition_broadcast(rb, rinv[0:1, :])
                    nc.vector.tensor_tensor(
                        out=oT[hp:hp + Dh, fc, :],
                        in0=o_ps[0:64, :], in1=rb,
                        op=OP.mult,
                    )

        # --------- Phases 4+5 merged: o-proj + residual + final LN ----------
        # Interleaved per token tile so the LN/transpose chain (which gates
        # FFN1) starts as soon as each x tile lands, instead of after all of
        # phase 4.
        with (
            tc.tile_pool(name="hT", bufs=1) as hTp,
            tc.tile_pool(name="w2l", bufs=3) as w2pool,
            tc.tile_pool(name="w1l", bufs=4) as w1pool,
        ):
            hT = hTp.tile([P, NFFC, Sq], BF16, tag="hT")
            hTq = hTp.tile([P, NFFC, Sq], E4, tag="hTq")
            hTd = hTp.tile([P, NFFC, Sq], E4, tag="hTd")

            # w1/w2 prefetch: first chunks load during the attention tail
            w1tiles = []
            for g in range(2):
                w1c = w1pool.tile([P, NFC, 2, 256], E4, tag="w1c", name="w1c")
                nc.gpsimd.dma_start(w1c, w1_d[g])
                w1tiles.append(w1c)
            w2tiles = []
            for g in range(2):
                w2c = w2pool.tile([P, 4, 2, H], E4, tag="w2c", name="w2c")
                nc.gpsimd.dma_start(w2c, w2_d[g])
                w2tiles.append(w2c)

            with tc.tile_pool(name="y3", bufs=1) as y3p:
                y3T = y3p.tile([P, NFC, Sq], E4, tag="y3T")
                y3D = y3p.tile([P, NFC, Sq], E4, tag="y3D")
                # one scope for phases 4+5 AND FFN1: f1ps coexists with the
                # ph45 PSUM pools (2+2+4 = 8 banks), so FFN1's matmuls start
                # the moment y3's first token block lands instead of waiting
                # for the ph45 pools to drain
                with (
                    tc.tile_pool(name="ph5", bufs=4) as ph5,
                    tc.tile_pool(name="ph5w", bufs=3) as ph5w,
                    tc.tile_pool(name="ph5c", bufs=1) as ph5c,
                    tc.tile_pool(
                        name="ph4ps", bufs=2, space=bass.MemorySpace.PSUM
                    ) as ps4,
                    tc.tile_pool(
                        name="f1ps", bufs=3, space=bass.MemorySpace.PSUM
                    ) as f1ps,
                ):
                    eps_t = ph5c.tile([P, 1], F32, tag="eps")
                    nc.vector.memset(eps_t, EPS)
                    lnfg_b = lnfb_b = bo_b = None
                    if ln_affine:
                        lnfg_b = ph5c.tile([P, H], F32, tag="lnfg")
                        lnfb_b = ph5c.tile([P, H], F32, tag="lnfb")
                        nc.gpsimd.dma_start(lnfg_b, _pbcast(lnfg_d[:], P))
                        nc.gpsimd.dma_start(lnfb_b, _pbcast(lnfb_d[:], P))
                    if with_biases:
                        bo_b = ph5c.tile([P, H], F32, tag="bob")
                        nc.gpsimd.dma_start(bo_b, _pbcast(bo_d[:], P))
                    for t in range(NTC):
                        for half in range(2):
                            ps = ps4.tile([P, 512], F32, tag="mm", name="ps")
                            # preload 256*x2 (fp32 identity matmul), then
                            # accumulate 256*(o @ wo) on top; one ACT copy
                            # descales into x_sb — no DVE in the chain
                            nc.tensor.matmul(
                                ps, id256,
                                x2_sb[:, t, half * 512:(half + 1) * 512],
                                start=True, stop=False,
                            )
                            for c in range(NFC // 2):
                                _mmdr(
                                    ps,
                                    oT[:, 2 * c:2 * c + 2, t * P:(t + 1) * P],
                                    wo_sb[:, 2 * c:2 * c + 2,
                                          half * 512:(half + 1) * 512],
                                    start=False, stop=(c == NFC // 2 - 1),
                                )
                            xs = x_sb[:, t, half * 512:(half + 1) * 512]
                            nc.scalar.mul(xs, ps, 1.0 / (SO * SW))
                            if with_biases:
                                nc.vector.tensor_tensor(
                                    out=xs, in0=xs,
                                    in1=bo_b[:, half * 512:(half + 1) * 512],
                                    op=OP.add,
                                )
                        yt = ph5w.tile([P, H], BF16, tag="yt", name="yt")
                        _layer_norm(
                            ph5, yt, x_sb[:, t, :], lnfg_b, lnfb_b, eps_t
                        )
                        # DMA-xbar transpose + GPSIMD quantize: keeps the
                        # y3 chain off ACT (the phase's pacing engine) and
                        # off PSUM entirely
                        ytT = ph5w.tile([P, NFC, P], BF16, tag="ytT",
                                        name="ytT")
                        nc.sync.dma_start_transpose(ytT, yt)
                        yq = y3T[:, :, t * P:(t + 1) * P]
                        nc.gpsimd.tensor_copy(yq, ytT[:, :, :])
                        nc.vector.tensor_tensor(
                            out=y3D[:, :, t * P:(t + 1) * P],
                            in0=ytT[:, :, :], in1=yq, op=OP.subtract,
                        )

                    # FFN1 + gelu -> hT fully resident in SBUF
                    b1_sb = ph5c.tile([P, NFFC], F32, tag="b1")
                    if with_biases:
                        nc.gpsimd.dma_start(b1_sb, b1_d[:, :])
                    for g in range(NFFC // 2):
                        if g < 2:
                            w1c = w1tiles[g]
                        else:
                            w1c = w1pool.tile(
                                [P, NFC, 2, 256], E4, tag="w1c", name="w1c"
                            )
                            nc.sync.dma_start(w1c, w1_d[g])
                        ps = f1ps.tile([P, 2, Sq], F32, tag="mm", name="ps")
                        # 64*h = y3q@(w1q + w1d) + y3d@w1q  (3 DR sets).
                        # Token-tile-major so the first matmuls only need
                        # y3's first token block (start before phase 5 ends).
                        sets = [(0, y3T), (1, y3T), (0, y3D)]
                        for i in range(2):
                            for t in range(NTC):
                                for s, (ws, ya) in enumerate(sets):
                                    for c in range(NFC // 2):
                                        nc.tensor.matmul(
                                            ps[:, i, t * P:(t + 1) * P],
                                            w1c[:, 2 * c:2 * c + 2, ws,
                                                i * P:(i + 1) * P],
                                            ya[:, 2 * c:2 * c + 2,
                                               t * P:(t + 1) * P],
                                            start=(s == 0 and c == 0),
                                            stop=(s == 2 and
                                                  c == NFC // 2 - 1),
                                            perf_mode=DR,
                                        )
                        ffc = g * 2
                        if with_biases:
                            for i in range(2):
                                nc.scalar.activation(
                                    hT[:, ffc + i, :], ps[:, i, :], AF.Gelu,
                                    bias=b1_sb[:, ffc + i:ffc + i + 1],
                                    scale=1.0 / SW,
                                )
                        else:
                            nc.scalar.activation(
                                hT[:, ffc:ffc + 2, :], ps[:, :, :], AF.Gelu,
                                bias=0.0, scale=1.0 / SW,
                            )
                        # h as fp8 value + residual for corrected-fp8 FFN2
                        hq = hTq[:, ffc:ffc + 2, :]
                        nc.scalar.copy(hq, hT[:, ffc:ffc + 2, :])
                        nc.vector.tensor_tensor(
                            out=hTd[:, ffc:ffc + 2, :],
                            in0=hT[:, ffc:ffc + 2, :], in1=hq, op=OP.subtract,
                        )

            # FFN2: single pass, full 8-bank PSUM accumulation
            with (
                tc.tile_pool(name="ph7c", bufs=1) as ph7c,
                tc.tile_pool(name="outp", bufs=2) as outp,
                tc.tile_pool(
                    name="f2ps", bufs=1, space=bass.MemorySpace.PSUM
                ) as f2ps,
            ):
                b2_b = ph7c.tile([P, H], F32, tag="b2b")
                if with_biases:
                    nc.gpsimd.dma_start(b2_b, _pbcast(b2_d[:], P))
                acc = [
                    f2ps.tile([P, H], F32, tag=f"acc{t}", name=f"acc{t}")
                    for t in range(NTC)
                ]
                NG = NFFC // 4
                SETS2 = [(hTq, 0), (hTq, 1), (hTd, 0)]

                def _f2mm(w2c, g, jp, t, nt, first, last):
                    f0 = g * 4 + 2 * jp
                    for s, (ha, ws) in enumerate(SETS2):
                        nc.tensor.matmul(
                            acc[t][:, nt * 512:(nt + 1) * 512],
                            ha[:, f0:f0 + 2, t * P:(t + 1) * P],
                            w2c[:, 2 * jp:2 * jp + 2, ws,
                                nt * 512:(nt + 1) * 512],
                            start=(first and s == 0),
                            stop=(last and s == 2),
                            perf_mode=DR,
                        )

                w2cs = []
                for g in range(NG):
                    if g < 2:
                        w2c = w2tiles[g]
                    else:
                        w2c = w2pool.tile(
                            [P, 4, 2, H], E4, tag="w2c", name="w2c"
                        )
                        nc.sync.dma_start(w2c, w2_d[g])
                    w2cs.append(w2c)
                    if g < NG - 2:
                        for jp in range(2):
                            for t in range(NTC):
                                for nt in range(2):
                                    _f2mm(w2c, g, jp, t, nt,
                                          first=(g == 0 and jp == 0),
                                          last=False)
                # last two chunks token-tile-major: each tile's 24 matmuls
                # (~2.6us) fully hide the previous tile's epilogue chain
                # (descale + residual add + store on rotating DMA queues)
                dma_engines = [nc.sync, nc.gpsimd, nc.scalar, nc.sync]
                for t in range(NTC):
                    for g in (NG - 2, NG - 1):
                        for jp in range(2):
                            for nt in range(2):
                                _f2mm(w2cs[g], g, jp, t, nt, first=False,
                                      last=(g == NG - 1 and jp == 1))
                    ot = outp.tile([P, H], F32, tag="ot", name="ot")
                    tmp = outp.tile([P, H], F32, tag="tm", name="tm")
                    for nt in range(2):
                        sl = slice(nt * 512, (nt + 1) * 512)
                        nc.scalar.mul(
                            tmp[:, sl], acc[t][:, sl], 1.0 / SW
                        )
                        nc.vector.tensor_tensor(
                            out=ot[:, sl], in0=tmp[:, sl],
                            in1=x_sb[:, t, sl], op=OP.add,
                        )
                        if with_biases:
                            nc.vector.tensor_tensor(
                                out=ot[:, sl], in0=ot[:, sl],
                                in1=b2_b[:, sl], op=OP.add,
                            )
                        dma_engines[(2 * t + nt) % 4].dma_start(
                            out_d[t * P:(t + 1) * P, sl], ot[:, sl]
                        )

    nc.compile()
    return nc


_CACHE: dict = {}


def _get_program(ln_affine=True, with_biases=True):
    key = (ln_affine, with_biases)
    if key not in _CACHE:
        _CACHE[key] = build_program(
            ln_affine=ln_affine, with_biases=with_biases
        )
    return _CACHE[key]


def _detect_fast_flags(inputs):
    ones = lambda k: bool(np.all(np.asarray(inputs[k]) == 1.0))
    zeros = lambda k: bool(np.all(np.asarray(inputs[k]) == 0.0))
    ln_affine = not (
        ones("ln1_g") and ones("ln2_g") and ones("lnf_g")
        and zeros("ln1_b") and zeros("ln2_b") and zeros("lnf_b")
    )
    with_biases = not (
        zeros("bq") and zeros("bk") and zeros("bv") and zeros("bo")
        and zeros("b1") and zeros("b2")
    )
    return ln_affine, with_biases


def _make_in_maps(inputs: dict) -> list[dict]:
    np_e4 = mybir.dt.np(E4)
    f32 = lambda a: np.ascontiguousarray(np.asarray(a, dtype=np.float32))
    w8 = lambda a: np.ascontiguousarray(
        (np.asarray(a, dtype=np.float32) * SW).astype(np_e4)
    )

    def pack_corr(w, gsz, gdim):
        """fp8 value+residual stream: (G, P, K//P, 2, gsz) for a (K, N) w."""
        ws = np.asarray(w, dtype=np.float32) * SW
        wq = ws.astype(np_e4)
        wd = (ws - wq.astype(np.float32)).astype(np_e4)
        K, N = ws.shape
        G = N // gsz
        out = np.empty((G, P, K // P, 2, gsz), np_e4)
        for s, arr in enumerate((wq, wd)):
            # arr[(kc*P + p), g*gsz + c] -> out[g, p, kc, s, c]
            v = arr.reshape(K // P, P, G, gsz)
            out[:, :, :, s, :] = v.transpose(2, 1, 0, 3)
        return np.ascontiguousarray(out)

    def pack_corr2(w):
        """(NG, P, 4, 2, H) for w2 (FF, H): chunk rows, value+residual."""
        ws = np.asarray(w, dtype=np.float32) * SW
        wq = ws.astype(np_e4)
        wd = (ws - wq.astype(np.float32)).astype(np_e4)
        FFr, N = ws.shape
        NGl = FFr // 512
        out = np.empty((NGl, P, 4, 2, N), np_e4)
        for s, arr in enumerate((wq, wd)):
            # arr[g*512 + c*P + p, f] -> out[g, p, c, s, f]
            v = arr.reshape(NGl, 4, P, N)
            out[:, :, :, s, :] = v.transpose(0, 2, 1, 3)
        return np.ascontiguousarray(out)

    np_bf = mybir.dt.np(BF16)
    x1 = np.ascontiguousarray(
        np.asarray(inputs["x1"], dtype=np.float32).astype(np_bf)
    )
    x2 = f32(inputs["x2"])
    attn_bias = np.asarray(inputs["attn_bias"], dtype=np.float32)
    shared = {
        "wq": w8(inputs["wq"]),
        "wk": w8(inputs["wk"]),
        "wv": w8(inputs["wv"]),
        "wo": w8(inputs["wo"]),
        "bq_pc": f32(np.asarray(inputs["bq"]).reshape(NFC, P).T),
        "bk_pc": f32(np.asarray(inputs["bk"]).reshape(NFC, P).T),
        "bv4": f32(np.asarray(inputs["bv"], dtype=np.float32) * SV),
        "bo": f32(inputs["bo"]),
        "w1p": pack_corr(inputs["w1"], 256, None),
        "b1_pc": f32(np.asarray(inputs["b1"]).reshape(NFFC, P).T),
        "w2p": pack_corr2(inputs["w2"]),
        "b2": f32(inputs["b2"]),
        "ln1_g": f32(inputs["ln1_g"]),
        "ln1_b": f32(inputs["ln1_b"]),
        "ln2_g": f32(inputs["ln2_g"]),
        "ln2_b": f32(inputs["ln2_b"]),
        "lnf_g": f32(inputs["lnf_g"]),
        "lnf_b": f32(inputs["lnf_b"]),
    }
    in_maps = []
    for c in range(8):
        b, half = c // 2, c % 2
        q0 = half * Sq
        in_maps.append(
            {
                "x1": x1[b],
                "x2h": np.ascontiguousarray(x2[b, q0:q0 + Sq]),
                "biasT": np.ascontiguousarray(
                    (attn_bias[b, :, q0:q0 + Sq, :].transpose(0, 2, 1)
                     / SCALE).astype(np_e4)
                ),
                **shared,
            }
        )
    return in_maps


def _assemble(results: list[dict]) -> np.ndarray:
    out = np.empty((B, S, H), np.float32)
    for c in range(8):
        b, half = c // 2, c % 2
        out[b, half * Sq:(half + 1) * Sq] = results[c]["out"]
    return out


def run(inputs: dict, **run_kwargs):
    from concourse.bass_utils import run_bass_kernel_spmd

    ln_affine, with_biases = _detect_fast_flags(inputs)
    nc = _get_program(ln_affine=ln_affine, with_biases=with_biases)
    in_maps = _make_in_maps(inputs)
    res = run_bass_kernel_spmd(nc, in_maps, core_ids=list(range(8)), **run_kwargs)
    return _assemble(res.results), res


def kernel(**inputs) -> np.ndarray:
    out, _ = run(inputs)
    return out



# revision 8
# speedup vs baseline: 1.0135x; 1.0135x over previous
"""CrossTransformerLayer on 8 TRN2 NeuronCores.

Sharding: core c -> (batch b = c//2, q-half = c%2). Each core computes its
512 query rows of its batch end-to-end (k/v over the full 1024-token x1
sequence), so no cross-core collectives are needed. The host slices inputs,
pre-transposes the attention bias to [head, k, q], and reassembles the 8
output slices.

Precision plan (rel-err budget 2e-2; measured ~3e-3):
  - QKV/O projection weights and the attention p@v path run in fp8e4m3
    with DoubleRow perf mode (stationary+moving both fp8, contraction 256
    per matmul). Weights are host-prescaled x64 (descaled in the PSUM->SBUF
    copy); v is carried x4, exp(p) is carried /16, oT is carried x4, with
    the scales cancelling exactly through the rowsum trick.
  - Scores (q.k^T, contraction 64) and the FFN stay bf16: fp8 in the FFN
    measurably busts the error budget, and the score matmul cannot pack a
    128-row contraction.
  - Layernorm stats/softmax logits/residuals/output stay fp32.

Engine balance: exp() dominates attention (ACT engine), so the attention
bias lands in PSUM via identity-matmul preloads (PE), keeping DVE nearly
idle there; layernorm application and half of the transpose/projection
PSUM->SBUF copies run on ACT (Identity with per-partition scale/bias),
the other half on DVE.
"""

import math
import sys

sys.path.insert(0, "/opt/trn_rl_repo")

from contextlib import ExitStack

import numpy as np

import concourse.bass as bass
import concourse.tile as tile
from concourse import bacc, mybir
from concourse.masks import make_identity

F32 = mybir.dt.float32
BF16 = mybir.dt.bfloat16
E4 = mybir.dt.float8e4
DR = mybir.MatmulPerfMode.DoubleRow

B = 4
S = 1024   # full (k) sequence
Sq = 512   # query rows per core
H = 1024
NH = 16
Dh = 64    # head dim
FF = 4096
P = 128
NKT = S // P    # 8 k-token tiles
NFC = H // P    # 8 feature chunks
NTC = Sq // P   # 4 q-token tiles
NFFC = FF // P  # 32 ff chunks
EPS = 1e-5
SCALE = float(Dh) ** -0.5
SW = 64.0   # weight prescale (wq/wk/wv/wo stored as 64*w in fp8)
SV = 4.0    # v carried as 4*v in v_aug
SE = 16.0   # exp carried as e/16
SO = SV  # oT ends up carried x4 via the rowsum cancellation
AF = mybir.ActivationFunctionType
OP = mybir.AluOpType


def _pbcast(ap: bass.AP, parts: int) -> bass.AP:
    """[.., N] access pattern -> [parts, .., N] with partition step 0."""
    return bass.AP(
        tensor=ap.tensor,
        offset=ap.offset,
        ap=[[0, parts]] + [list(d) for d in ap.ap],
    )


def build_program(ln_affine=True, with_biases=True):
    nc = bacc.Bacc("TRN2", target_bir_lowering=False, debug=False)

    x1_d = nc.dram_tensor("x1", (S, H), BF16, kind="ExternalInput")
    x2h_d = nc.dram_tensor("x2h", (Sq, H), F32, kind="ExternalInput")
    biasT_d = nc.dram_tensor("biasT", (NH, S, Sq), E4, kind="ExternalInput")
    wq_d = nc.dram_tensor("wq", (H, H), E4, kind="ExternalInput")
    wk_d = nc.dram_tensor("wk", (H, H), E4, kind="ExternalInput")
    wv_d = nc.dram_tensor("wv", (H, H), E4, kind="ExternalInput")
    wo_d = nc.dram_tensor("wo", (H, H), E4, kind="ExternalInput")
    bq_d = nc.dram_tensor("bq_pc", (P, NFC), F32, kind="ExternalInput")
    bk_d = nc.dram_tensor("bk_pc", (P, NFC), F32, kind="ExternalInput")
    bv_d = nc.dram_tensor("bv4", (H,), F32, kind="ExternalInput")
    bo_d = nc.dram_tensor("bo", (H,), F32, kind="ExternalInput")
    # fp8 value+residual weight streams, host-packed into the exact SBUF
    # layout (one large-contiguous DMA per chunk): [g, p, kc, set, cols]
    w1_d = nc.dram_tensor(
        "w1p", (NFFC // 2, P, NFC, 2, 256), E4, kind="ExternalInput"
    )
    b1_d = nc.dram_tensor("b1_pc", (P, NFFC), F32, kind="ExternalInput")
    w2_d = nc.dram_tensor(
        "w2p", (NFFC // 4, P, 4, 2, H), E4, kind="ExternalInput"
    )
    b2_d = nc.dram_tensor("b2", (H,), F32, kind="ExternalInput")
    ln1g_d = nc.dram_tensor("ln1_g", (H,), F32, kind="ExternalInput")
    ln1b_d = nc.dram_tensor("ln1_b", (H,), F32, kind="ExternalInput")
    ln2g_d = nc.dram_tensor("ln2_g", (H,), F32, kind="ExternalInput")
    ln2b_d = nc.dram_tensor("ln2_b", (H,), F32, kind="ExternalInput")
    lnfg_d = nc.dram_tensor("lnf_g", (H,), F32, kind="ExternalInput")
    lnfb_d = nc.dram_tensor("lnf_b", (H,), F32, kind="ExternalInput")
    out_d = nc.dram_tensor("out", (Sq, H), F32, kind="ExternalOutput")

    def _mmdr(out, lhsT, rhs, start, stop):
        nc.tensor.matmul(out, lhsT, rhs, start=start, stop=stop, perf_mode=DR)

    def _layer_norm(pool, y_out, x_in, g_b, b_b, eps_t, apply_on="act"):
        """y = (x - mean)/sqrt(var+eps) [* g + b] on a [128, H] tile.

        Stats on DVE; apply on ACT (Identity with per-partition scale/bias)
        or on GPSIMD (tensor_scalar) when ACT is the busier engine.
        """
        stats = pool.tile([P, 2, 6], F32, tag="ln_stats", name="stats")
        nc.vector.bn_stats(stats[:, 0, :], x_in[:, 0:512])
        nc.vector.bn_stats(stats[:, 1, :], x_in[:, 512:1024])
        mv = pool.tile([P, 2], F32, tag="ln_mv", name="mv")
        nc.vector.bn_aggr(mv, stats)
        # rstd = 1/sqrt(var + eps): one ACT Sqrt + a tiny DVE reciprocal.
        # Sqrt/Identity/Copy share one table set, so LN regions cost a
        # single LoadActFuncSet instead of the Ln/Exp-chain's per-call set
        # thrash (~1.3us per reload).
        std = pool.tile([P, 1], F32, tag="ln_std", name="std")
        nc.scalar.activation(std, mv[:, 1:2], AF.Sqrt, bias=eps_t, scale=1.0)
        rstd = pool.tile([P, 1], F32, tag="ln_rstd", name="rstd")
        nc.vector.reciprocal(rstd, std)
        if apply_on == "pool":
            # direct (x - m) * r on GPSIMD — no negmr precompute needed
            nc.gpsimd.tensor_scalar(
                y_out, x_in, mv[:, 0:1], rstd[:, 0:1],
                op0=OP.subtract, op1=OP.mult,
            )
        else:
            negmr = pool.tile([P, 1], F32, tag="ln_negmr", name="negmr")
            nc.vector.tensor_scalar(
                negmr, mv[:, 0:1], rstd[:, 0:1], -1.0, op0=OP.mult, op1=OP.mult
            )
            nc.scalar.activation(
                y_out, x_in, AF.Identity, bias=negmr[:, 0:1], scale=rstd[:, 0:1]
            )
        if ln_affine:
            nc.vector.tensor_mul(y_out, y_out, g_b)
            nc.vector.tensor_add(y_out, y_out, b_b)

    with tile.TileContext(nc) as tc, ExitStack() as top:
        persist = top.enter_context(tc.tile_pool(name="persist", bufs=1))
        ident = persist.tile([P, P], BF16, tag="ident")
        make_identity(nc, ident)
        # fp32 identity scaled by SO*SW: preloads 256*x2 into the o-proj PSUM
        # so the residual add rides the matmul accumulation instead of DVE
        id256 = persist.tile([P, P], F32, tag="id256")
        nc.vector.tensor_scalar(id256, ident, float(SO * SW), None, op0=OP.mult)
        # fp8 DoubleRow selector identities: idDR[j][:, j, :] = I, other
        # slot 0. Lets the attention-bias PSUM preload run as a DR matmul
        # (0.5 cycles/row instead of bf16's 1.0), halving its PE time.
        idDR = []
        for j in range(2):
            t = persist.tile([P, 2, P], E4, tag=f"idDR{j}")
            nc.vector.memset(t, 0.0)
            nc.vector.tensor_copy(t[:, j, :], ident)
            idDR.append(t)
        oT = persist.tile([P, NFC, Sq], E4, tag="oT")       # 4*o_norm [p,fc,q]

        xp = top.enter_context(tc.tile_pool(name="xp", bufs=1))
        x_sb = xp.tile([P, NTC, H], F32, tag="x")   # [p, tc, f], ph 4-7
        x2_sb = xp.tile([P, NTC, H], F32, tag="x2")  # resident x2h, ph 1-4
        w4pool = top.enter_context(tc.tile_pool(name="ph4w", bufs=1))
        with (
            tc.tile_pool(name="qkv", bufs=1) as qkvp,           # phases 1-3
            tc.tile_pool(name="bias_s", bufs=2) as bpool,
            tc.tile_pool(name="expp", bufs=4) as epool,
            tc.tile_pool(name="rin", bufs=2) as rpool,
        ):
            qT = qkvp.tile([P, NFC, Sq], BF16, tag="qT")        # [p, fc, q]
            kT = qkvp.tile([P, NFC, S], BF16, tag="kT")         # [p, fc, k]
            v_aug = qkvp.tile([P, NKT, NH * 65], E4, tag="vaug")

            # ------------ Phase 1+2: LN, transpose, QKV projections ---------
            # y1T lives through attention: the k-projection is interleaved
            # into the head loop so softmax exp (the attention bottleneck)
            # can start as soon as the first kT chunk lands.
            y1T = qkvp.tile([P, NFC, S], E4, tag="y1T")
            y2T = qkvp.tile([P, NFC, Sq], E4, tag="y2T")
            wk_sb = qkvp.tile([P, NFC, H], E4, tag="wk")
            if True:
                with (
                    tc.tile_pool(name="ph1", bufs=4) as ph1,
                    tc.tile_pool(name="ph1x", bufs=NKT) as ph1x,
                    tc.tile_pool(name="trp", bufs=4) as trp,
                    tc.tile_pool(name="ph1w", bufs=4) as ph1w,
                    tc.tile_pool(name="ph1c", bufs=1) as ph1c,
                    tc.tile_pool(name="wload", bufs=1) as wpool,
                    tc.tile_pool(name="vecs", bufs=1) as vecs,
                    tc.tile_pool(
                        name="ph2ps", bufs=3, space=bass.MemorySpace.PSUM
                    ) as ps2,
                ):
                    eps_t = ph1c.tile([P, 1], F32, tag="eps")
                    nc.vector.memset(eps_t, EPS)
                    ln1g_b = ln1b_b = ln2g_b = ln2b_b = None
                    if ln_affine:
                        ln1g_b = ph1c.tile([P, H], F32, tag="ln1g")
                        ln1b_b = ph1c.tile([P, H], F32, tag="ln1b")
                        ln2g_b = ph1c.tile([P, H], F32, tag="ln2g")
                        ln2b_b = ph1c.tile([P, H], F32, tag="ln2b")
                        nc.gpsimd.dma_start(ln1g_b, _pbcast(ln1g_d[:], P))
                        nc.gpsimd.dma_start(ln1b_b, _pbcast(ln1b_d[:], P))
                        nc.gpsimd.dma_start(ln2g_b, _pbcast(ln2g_d[:], P))
                        nc.gpsimd.dma_start(ln2b_b, _pbcast(ln2b_d[:], P))

                    def _w_full(wd, name, tag):
                        w_sb = wpool.tile([P, NFC, H], E4, tag=tag, name=name)
                        nc.gpsimd.dma_start(
                            w_sb, wd.rearrange("(kc p) f -> p kc f", p=P)
                        )
                        return w_sb

                    bq_sb = bv_b = None
                    if with_biases:
                        bq_sb = vecs.tile([P, NFC], F32, tag="bq")
                        bk_sb = qkvp.tile([P, NFC], F32, tag="bk")
                        bv_b = vecs.tile([P, H], F32, tag="bvb")
                        nc.gpsimd.dma_start(bq_sb, bq_d[:, :])
                        nc.gpsimd.dma_start(bk_sb, bk_d[:, :])
                        nc.gpsimd.dma_start(bv_b, _pbcast(bv_d[:], P))

                    # ones columns of v_aug (slot 64 of each head) = 1.0
                    ones_view = v_aug[:, :, :].rearrange(
                        "p t (h j) -> p t h j", j=65
                    )[:, :, :, 64:65]
                    nc.vector.memset(ones_view, 1.0)

                    wv_sb = _w_full(wv_d[:, :], "wv_sb", tag="wv")

                    xp_i = [0]

                    def _xpose_block(yt, dst_slice, on_act):
                        """DMA-xbar transpose of a [P, H] bf16 tile into a
                        bf16 staging tile (runs on the DMA engines), then one
                        quantizing copy to the fp8 destination. Keeps PE, DVE
                        and ACT free of transpose traffic."""
                        ytT = trp.tile([P, NFC, P], BF16, tag="ytT", name="ytT")
                        nc.sync.dma_start_transpose(ytT, yt)
                        i = xp_i[0]
                        xp_i[0] += 1
                        if i % 3 == 2:
                            nc.scalar.copy(dst_slice, ytT[:, :, :])
                        elif i % 3 == 1:
                            nc.vector.tensor_copy(dst_slice, ytT[:, :, :])
                        else:
                            nc.gpsimd.tensor_copy(dst_slice, ytT[:, :, :])

                    for t in range(NTC):  # x2h -> y2 -> y2T (x2h kept resident)
                        eng = nc.sync if t % 2 == 0 else nc.scalar
                        eng.dma_start(
                            x2_sb[:, t, :], x2h_d[t * P:(t + 1) * P, :]
                        )
                        yt = ph1w.tile([P, H], BF16, tag="yt", name="yt")
                        _layer_norm(ph1, yt, x2_sb[:, t, :], ln2g_b, ln2b_b, eps_t)
                        _xpose_block(
                            yt, y2T[:, :, t * P:(t + 1) * P], on_act=(t % 2 == 1)
                        )

                    # pre-issue all x1 loads, split over the SP and Pool DMA
                    # queues (per-queue issue is serialized in-order)
                    x1t = []
                    for t in range(NKT):
                        xt = ph1x.tile([P, H], BF16, tag="x1t", name="xt")
                        eng = nc.sync if t % 2 == 0 else nc.gpsimd
                        eng.dma_start(xt, x1_d[t * P:(t + 1) * P, :])
                        x1t.append(xt)

                    wq_sb = _w_full(wq_d[:, :], "wq_sb", tag="wq")
                    nc.gpsimd.dma_start(
                        wk_sb, wk_d[:, :].rearrange("(kc p) f -> p kc f", p=P)
                    )

                    # q^T[fo, :] = sum_c wq[pair c, fo].T @ y2T[pair c]  (DR)
                    for fp in range(NFC // 2):
                        ps = ps2.tile([P, 2, Sq], F32, tag="mm", name="ps")
                        for i in range(2):
                            fo = 2 * fp + i
                            for c in range(NFC // 2):
                                _mmdr(
                                    ps[:, i, :],
                                    wq_sb[:, 2 * c:2 * c + 2,
                                          fo * P:(fo + 1) * P],
                                    y2T[:, 2 * c:2 * c + 2, :],
                                    start=(c == 0), stop=(c == NFC // 2 - 1),
                                )
                        dst = qT[:, 2 * fp:2 * fp + 2, :]
                        if with_biases:
                            for i in range(2):
                                fo = 2 * fp + i
                                nc.vector.tensor_scalar(
                                    qT[:, fo, :], ps[:, i, :], 1.0 / SW,
                                    bq_sb[:, fo:fo + 1],
                                    op0=OP.mult, op1=OP.add,
                                )
                        elif fp % 2 == 0:
                            nc.vector.tensor_scalar(
                                dst, ps[:, :, :], 1.0 / SW, None, op0=OP.mult
                            )
                        else:
                            nc.scalar.mul(dst, ps[:, :, :], 1.0 / SW)

                    # x1 -> y1 -> y1T, interleaved with v[t] = y1T[t].T @ wv
                    for t in range(NKT):
                        yt = ph1w.tile([P, H], BF16, tag="yt", name="yt")
                        _layer_norm(ph1, yt, x1t[t], ln1g_b, ln1b_b, eps_t,
                                    apply_on=("pool" if t >= 2 else "act"))
                        _xpose_block(
                            yt, y1T[:, :, t * P:(t + 1) * P], on_act=(t % 2 == 1)
                        )
                        ps = ps2.tile([P, 2, 512], F32, tag="mm", name="ps")
                        for nt in range(2):
                            for c in range(NFC // 2):
                                _mmdr(
                                    ps[:, nt, :],
                                    y1T[:, 2 * c:2 * c + 2, t * P:(t + 1) * P],
                                    wv_sb[:, 2 * c:2 * c + 2,
                                          nt * 512:(nt + 1) * 512],
                                    start=(c == 0), stop=(c == NFC // 2 - 1),
                                )
                        dst = v_aug[:, t, :].rearrange(
                            "p (h j) -> p h j", j=65
                        )[:, :, 0:64]
                        src = ps.rearrange("p two (h j) -> p (two h) j", j=64)
                        if with_biases:
                            tmp = ph1w.tile(
                                [P, 2, 512], F32, tag="vtmp", name="vtmp"
                            )
                            nc.vector.tensor_scalar(
                                tmp, ps, SV / SW, None, op0=OP.mult
                            )
                            nc.vector.tensor_tensor(
                                out=dst,
                                in0=tmp.rearrange(
                                    "p two (h j) -> p (two h) j", j=64
                                ),
                                in1=bv_b[:, :].rearrange(
                                    "p (h j) -> p h j", j=64
                                ),
                                op=OP.add,
                            )
                        elif t % 2 == 0:
                            nc.vector.tensor_scalar(
                                dst, src, SV / SW, None, op0=OP.mult
                            )
                        else:
                            nc.scalar.mul(dst, src, SV / SW)

                    # k^T[fo, :] = sum_c wk[pair c, fo].T @ y1T[pair c]
                    for fo in range(NFC):
                        ps = ps2.tile([P, 2, 512], F32, tag="mm", name="ps")
                        for nt in range(2):
                            for c in range(NFC // 2):
                                _mmdr(
                                    ps[:, nt, :],
                                    wk_sb[:, 2 * c:2 * c + 2,
                                          fo * P:(fo + 1) * P],
                                    y1T[:, 2 * c:2 * c + 2,
                                        nt * 512:(nt + 1) * 512],
                                    start=(c == 0), stop=(c == NFC // 2 - 1),
                                )
                        dst = kT[:, fo, :]
                        if with_biases:
                            nc.vector.tensor_scalar(
                                dst, ps[:, :, :], 1.0 / SW, bk_sb[:, fo:fo + 1],
                                op0=OP.mult, op1=OP.add,
                            )
                        elif fo % 2 == 0:
                            nc.vector.tensor_scalar(
                                dst, ps[:, :, :], 1.0 / SW, None, op0=OP.mult
                            )
                        else:
                            nc.scalar.mul(dst, ps[:, :, :], 1.0 / SW)


            # wo prefetch (consumed in phase 4, loads during attention)
            wo_sb = w4pool.tile([P, NFC, H], E4, tag="wo")
            nc.gpsimd.dma_start(
                wo_sb, wo_d[:, :].rearrange("(kc p) f -> p kc f", p=P)
            )

            # ---------------- Phase 3: attention ----------------
            # Host pre-multiplies biasT by 1/scale (x8, fp8); identity-matmul
            # preloads (PE) land it in PSUM under the bf16 score matmul.
            # One Exp per 2-bank PSUM tile computes e/16 in fp8; o and the
            # rowsum accumulate via fp8 DoubleRow over kt pairs; the /16 and
            # v's x4 cancel in the rowsum division, leaving oT = 4*o_norm.
            with (
                tc.tile_pool(name="ph3c", bufs=1) as ph3c,
                tc.tile_pool(
                    name="sc_ps", bufs=3, space=bass.MemorySpace.PSUM
                ) as scps,
                tc.tile_pool(
                    name="o_ps", bufs=2, space=bass.MemorySpace.PSUM
                ) as ops,
            ):
                eln_t = ph3c.tile([P, 1], F32, tag="eln")
                nc.vector.memset(eln_t, -math.log(SE))
                for h in range(NH):
                    hp = (h % 2) * Dh
                    fc = h // 2
                    o_ps = ops.tile([65, Sq], F32, tag="o", name="o_ps")
                    bt = bpool.tile([P, NKT, Sq], E4, tag="bt", name="bt")
                    nc.sync.dma_start(
                        bt, biasT_d[h].rearrange("(kt p) q -> p kt q", p=P)
                    )
                    for kp in range(NKT // 2):
                        sc = scps.tile([P, 2, Sq], F32, tag="sc", name="sc_ps")
                        for j in range(2):
                            kt = 2 * kp + j
                            _mmdr(
                                sc[:, j, :], idDR[j],
                                bt[:, 2 * kp:2 * kp + 2, :],
                                start=True, stop=False,
                            )
                            nc.tensor.matmul(
                                sc[:, j, :],
                                kT[hp:hp + Dh, fc, kt * P:(kt + 1) * P],
                                qT[hp:hp + Dh, fc, :],
                                start=False, stop=True,
                            )
                        e2 = epool.tile([P, 2, Sq], E4, tag="expT", name="e2")
                        nc.scalar.activation(
                            e2, sc[:, :, :], AF.Exp,
                            bias=eln_t[:, 0:1], scale=SCALE,
                        )
                        _mmdr(
                            o_ps,
                            v_aug[:, 2 * kp:2 * kp + 2, h * 65:(h + 1) * 65],
                            e2[:, :, :],
                            start=(kp == 0), stop=(kp == NKT // 2 - 1),
                        )
                    rinv = rpool.tile([1, Sq], F32, tag="rinv", name="rinv")
                    nc.vector.reciprocal(rinv, o_ps[64:65, :])
                    rb = rpool.tile([Dh, Sq], F32, tag="rb", name="rb")
                    nc.gpsimd.partition_broadcast(rb, rinv[0:1, :])
                    nc.vector.tensor_tensor(
                        out=oT[hp:hp + Dh, fc, :],
                        in0=o_ps[0:64, :], in1=rb,
                        op=OP.mult,
                    )

        # --------- Phases 4+5 merged: o-proj + residual + final LN ----------
        # Interleaved per token tile so the LN/transpose chain (which gates
        # FFN1) starts as soon as each x tile lands, instead of after all of
        # phase 4.
        with (
            tc.tile_pool(name="hT", bufs=1) as hTp,
            tc.tile_pool(name="w2l", bufs=3) as w2pool,
            tc.tile_pool(name="w1l", bufs=4) as w1pool,
        ):
            hT = hTp.tile([P, NFFC, Sq], BF16, tag="hT")
            hTq = hTp.tile([P, NFFC, Sq], E4, tag="hTq")
            hTd = hTp.tile([P, NFFC, Sq], E4, tag="hTd")

            # w1/w2 prefetch: first chunks load during the attention tail
            w1tiles = []
            for g in range(2):
                w1c = w1pool.tile([P, NFC, 2, 256], E4, tag="w1c", name="w1c")
                nc.gpsimd.dma_start(w1c, w1_d[g])
                w1tiles.append(w1c)
            w2tiles = []
            for g in range(2):
                w2c = w2pool.tile([P, 4, 2, H], E4, tag="w2c", name="w2c")
                nc.gpsimd.dma_start(w2c, w2_d[g])
                w2tiles.append(w2c)

            with tc.tile_pool(name="y3", bufs=1) as y3p:
                y3T = y3p.tile([P, NFC, Sq], E4, tag="y3T")
                y3D = y3p.tile([P, NFC, Sq], E4, tag="y3D")
                # one scope for phases 4+5 AND FFN1: f1ps coexists with the
                # ph45 PSUM pools (2+2+4 = 8 banks), so FFN1's matmuls start
                # the moment y3's first token block lands instead of waiting
                # for the ph45 pools to drain
                with (
                    tc.tile_pool(name="ph5", bufs=4) as ph5,
                    tc.tile_pool(name="ph5w", bufs=3) as ph5w,
                    tc.tile_pool(name="ph5c", bufs=1) as ph5c,
                    tc.tile_pool(
                        name="ph4ps", bufs=2, space=bass.MemorySpace.PSUM
                    ) as ps4,
                    tc.tile_pool(
                        name="f1ps", bufs=3, space=bass.MemorySpace.PSUM
                    ) as f1ps,
                ):
                    eps_t = ph5c.tile([P, 1], F32, tag="eps")
                    nc.vector.memset(eps_t, EPS)
                    lnfg_b = lnfb_b = bo_b = None
                    if ln_affine:
                        lnfg_b = ph5c.tile([P, H], F32, tag="lnfg")
                        lnfb_b = ph5c.tile([P, H], F32, tag="lnfb")
                        nc.gpsimd.dma_start(lnfg_b, _pbcast(lnfg_d[:], P))
                        nc.gpsimd.dma_start(lnfb_b, _pbcast(lnfb_d[:], P))
                    if with_biases:
                        bo_b = ph5c.tile([P, H], F32, tag="bob")
                        nc.gpsimd.dma_start(bo_b, _pbcast(bo_d[:], P))
                    for t in range(NTC):
                        for half in range(2):
                            ps = ps4.tile([P, 512], F32, tag="mm", name="ps")
                            # preload 256*x2 (fp32 identity matmul), then
                            # accumulate 256*(o @ wo) on top; one ACT copy
                            # descales into x_sb — no DVE in the chain
                            nc.tensor.matmul(
                                ps, id256,
                                x2_sb[:, t, half * 512:(half + 1) * 512],
                                start=True, stop=False,
                            )
                            for c in range(NFC // 2):
                                _mmdr(
                                    ps,
                                    oT[:, 2 * c:2 * c + 2, t * P:(t + 1) * P],
                                    wo_sb[:, 2 * c:2 * c + 2,
                                          half * 512:(half + 1) * 512],
                                    start=False, stop=(c == NFC // 2 - 1),
                                )
                            xs = x_sb[:, t, half * 512:(half + 1) * 512]
                            nc.scalar.mul(xs, ps, 1.0 / (SO * SW))
                            if with_biases:
                                nc.vector.tensor_tensor(
                                    out=xs, in0=xs,
                                    in1=bo_b[:, half * 512:(half + 1) * 512],
                                    op=OP.add,
                                )
                        yt = ph5w.tile([P, H], BF16, tag="yt", name="yt")
                        _layer_norm(
                            ph5, yt, x_sb[:, t, :], lnfg_b, lnfb_b, eps_t
                        )
                        # DMA-xbar transpose + GPSIMD quantize: keeps the
                        # y3 chain off ACT (the phase's pacing engine) and
                        # off PSUM entirely
                        ytT = ph5w.tile([P, NFC, P], BF16, tag="ytT",
                                        name="ytT")
                        nc.sync.dma_start_transpose(ytT, yt)
                        yq = y3T[:, :, t * P:(t + 1) * P]
                        nc.gpsimd.tensor_copy(yq, ytT[:, :, :])
                        nc.vector.tensor_tensor(
                            out=y3D[:, :, t * P:(t + 1) * P],
                            in0=ytT[:, :, :], in1=yq, op=OP.subtract,
                        )

                    # FFN1 + gelu -> hT fully resident in SBUF
                    b1_sb = ph5c.tile([P, NFFC], F32, tag="b1")
                    if with_biases:
                        nc.gpsimd.dma_start(b1_sb, b1_d[:, :])
                    for g in range(NFFC // 2):
                        if g < 2:
                            w1c = w1tiles[g]
                        else:
                            w1c = w1pool.tile(
                                [P, NFC, 2, 256], E4, tag="w1c", name="w1c"
                            )
                            nc.sync.dma_start(w1c, w1_d[g])
                        ps = f1ps.tile([P, 2, Sq], F32, tag="mm", name="ps")
                        # 64*h = y3q@(w1q + w1d) + y3d@w1q  (3 DR sets).
                        # Full-width (512-token) matmuls: the PE sequencer
                        # (~86ns/matmul dispatch) is the binding resource, so
                        # fewer, larger matmuls beat token-tile pipelining.
                        sets = [(0, y3T), (1, y3T), (0, y3D)]
                        for i in range(2):
                            for s, (ws, ya) in enumerate(sets):
                                for c in range(NFC // 2):
                                    nc.tensor.matmul(
                                        ps[:, i, :],
                                        w1c[:, 2 * c:2 * c + 2, ws,
                                            i * P:(i + 1) * P],
                                        ya[:, 2 * c:2 * c + 2, :],
                                        start=(s == 0 and c == 0),
                                        stop=(s == 2 and
                                              c == NFC // 2 - 1),
                                        perf_mode=DR,
                                    )
                        ffc = g * 2
                        if with_biases:
                            for i in range(2):
                                nc.scalar.activation(
                                    hT[:, ffc + i, :], ps[:, i, :], AF.Gelu,
                                    bias=b1_sb[:, ffc + i:ffc + i + 1],
                                    scale=1.0 / SW,
                                )
                        else:
                            nc.scalar.activation(
                                hT[:, ffc:ffc + 2, :], ps[:, :, :], AF.Gelu,
                                bias=0.0, scale=1.0 / SW,
                            )
                        # h as fp8 value + residual for corrected-fp8 FFN2
                        hq = hTq[:, ffc:ffc + 2, :]
                        nc.scalar.copy(hq, hT[:, ffc:ffc + 2, :])
                        nc.vector.tensor_tensor(
                            out=hTd[:, ffc:ffc + 2, :],
                            in0=hT[:, ffc:ffc + 2, :], in1=hq, op=OP.subtract,
                        )

            # FFN2: single pass, full 8-bank PSUM accumulation
            with (
                tc.tile_pool(name="ph7c", bufs=1) as ph7c,
                tc.tile_pool(name="outp", bufs=2) as outp,
                tc.tile_pool(
                    name="f2ps", bufs=1, space=bass.MemorySpace.PSUM
                ) as f2ps,
            ):
                b2_b = ph7c.tile([P, H], F32, tag="b2b")
                if with_biases:
                    nc.gpsimd.dma_start(b2_b, _pbcast(b2_d[:], P))
                acc = [
                    f2ps.tile([P, H], F32, tag=f"acc{t}", name=f"acc{t}")
                    for t in range(NTC)
                ]
                NG = NFFC // 4
                SETS2 = [(hTq, 0), (hTq, 1), (hTd, 0)]

                def _f2mm(w2c, g, jp, t, nt, first, last):
                    f0 = g * 4 + 2 * jp
                    for s, (ha, ws) in enumerate(SETS2):
                        nc.tensor.matmul(
                            acc[t][:, nt * 512:(nt + 1) * 512],
                            ha[:, f0:f0 + 2, t * P:(t + 1) * P],
                            w2c[:, 2 * jp:2 * jp + 2, ws,
                                nt * 512:(nt + 1) * 512],
                            start=(first and s == 0),
                            stop=(last and s == 2),
                            perf_mode=DR,
                        )

                w2cs = []
                for g in range(NG):
                    if g < 2:
                        w2c = w2tiles[g]
                    else:
                        w2c = w2pool.tile(
                            [P, 4, 2, H], E4, tag="w2c", name="w2c"
                        )
                        nc.sync.dma_start(w2c, w2_d[g])
                    w2cs.append(w2c)
                    if g < NG - 2:
                        for jp in range(2):
                            for t in range(NTC):
                                for nt in range(2):
                                    _f2mm(w2c, g, jp, t, nt,
                                          first=(g == 0 and jp == 0),
                                          last=False)
                # last two chunks token-tile-major: each tile's 24 matmuls
                # (~2.6us) fully hide the previous tile's epilogue chain
                # (descale + residual add + store on rotating DMA queues)
                dma_engines = [nc.sync, nc.gpsimd, nc.scalar, nc.sync]
                for t in range(NTC):
                    for g in (NG - 2, NG - 1):
                        for jp in range(2):
                            for nt in range(2):
                                _f2mm(w2cs[g], g, jp, t, nt, first=False,
                                      last=(g == NG - 1 and jp == 1))
                    ot = outp.tile([P, H], F32, tag="ot", name="ot")
                    tmp = outp.tile([P, H], F32, tag="tm", name="tm")
                    for nt in range(2):
                        sl = slice(nt * 512, (nt + 1) * 512)
                        nc.scalar.mul(
                            tmp[:, sl], acc[t][:, sl], 1.0 / SW
                        )
                        nc.vector.tensor_tensor(
                            out=ot[:, sl], in0=tmp[:, sl],
                            in1=x_sb[:, t, sl], op=OP.add,
                        )
                        if with_biases:
                            nc.vector.tensor_tensor(
                                out=ot[:, sl], in0=ot[:, sl],
                                in1=b2_b[:, sl], op=OP.add,
                            )
                        dma_engines[(2 * t + nt) % 4].dma_start(
                            out_d[t * P:(t + 1) * P, sl], ot[:, sl]
                        )

    nc.compile()
    return nc


_CACHE: dict = {}


def _get_program(ln_affine=True, with_biases=True):
    key = (ln_affine, with_biases)
    if key not in _CACHE:
        _CACHE[key] = build_program(
            ln_affine=ln_affine, with_biases=with_biases
        )
    return _CACHE[key]


def _detect_fast_flags(inputs):
    ones = lambda k: bool(np.all(np.asarray(inputs[k]) == 1.0))
    zeros = lambda k: bool(np.all(np.asarray(inputs[k]) == 0.0))
    ln_affine = not (
        ones("ln1_g") and ones("ln2_g") and ones("lnf_g")
        and zeros("ln1_b") and zeros("ln2_b") and zeros("lnf_b")
    )
    with_biases = not (
        zeros("bq") and zeros("bk") and zeros("bv") and zeros("bo")
        and zeros("b1") and zeros("b2")
    )
    return ln_affine, with_biases


def _make_in_maps(inputs: dict) -> list[dict]:
    np_e4 = mybir.dt.np(E4)
    f32 = lambda a: np.ascontiguousarray(np.asarray(a, dtype=np.float32))
    w8 = lambda a: np.ascontiguousarray(
        (np.asarray(a, dtype=np.float32) * SW).astype(np_e4)
    )

    def pack_corr(w, gsz, gdim):
        """fp8 value+residual stream: (G, P, K//P, 2, gsz) for a (K, N) w."""
        ws = np.asarray(w, dtype=np.float32) * SW
        wq = ws.astype(np_e4)
        wd = (ws - wq.astype(np.float32)).astype(np_e4)
        K, N = ws.shape
        G = N // gsz
        out = np.empty((G, P, K // P, 2, gsz), np_e4)
        for s, arr in enumerate((wq, wd)):
            # arr[(kc*P + p), g*gsz + c] -> out[g, p, kc, s, c]
            v = arr.reshape(K // P, P, G, gsz)
            out[:, :, :, s, :] = v.transpose(2, 1, 0, 3)
        return np.ascontiguousarray(out)

    def pack_corr2(w):
        """(NG, P, 4, 2, H) for w2 (FF, H): chunk rows, value+residual."""
        ws = np.asarray(w, dtype=np.float32) * SW
        wq = ws.astype(np_e4)
        wd = (ws - wq.astype(np.float32)).astype(np_e4)
        FFr, N = ws.shape
        NGl = FFr // 512
        out = np.empty((NGl, P, 4, 2, N), np_e4)
        for s, arr in enumerate((wq, wd)):
            # arr[g*512 + c*P + p, f] -> out[g, p, c, s, f]
            v = arr.reshape(NGl, 4, P, N)
            out[:, :, :, s, :] = v.transpose(0, 2, 1, 3)
        return np.ascontiguousarray(out)

    np_bf = mybir.dt.np(BF16)
    x1 = np.ascontiguousarray(
        np.asarray(inputs["x1"], dtype=np.float32).astype(np_bf)
    )
    x2 = f32(inputs["x2"])
    attn_bias = np.asarray(inputs["attn_bias"], dtype=np.float32)
    shared = {
        "wq": w8(inputs["wq"]),
        "wk": w8(inputs["wk"]),
        "wv": w8(inputs["wv"]),
        "wo": w8(inputs["wo"]),
        "bq_pc": f32(np.asarray(inputs["bq"]).reshape(NFC, P).T),
        "bk_pc": f32(np.asarray(inputs["bk"]).reshape(NFC, P).T),
        "bv4": f32(np.asarray(inputs["bv"], dtype=np.float32) * SV),
        "bo": f32(inputs["bo"]),
        "w1p": pack_corr(inputs["w1"], 256, None),
        "b1_pc": f32(np.asarray(inputs["b1"]).reshape(NFFC, P).T),
        "w2p": pack_corr2(inputs["w2"]),
        "b2": f32(inputs["b2"]),
        "ln1_g": f32(inputs["ln1_g"]),
        "ln1_b": f32(inputs["ln1_b"]),
        "ln2_g": f32(inputs["ln2_g"]),
        "ln2_b": f32(inputs["ln2_b"]),
        "lnf_g": f32(inputs["lnf_g"]),
        "lnf_b": f32(inputs["lnf_b"]),
    }
    in_maps = []
    for c in range(8):
        b, half = c // 2, c % 2
        q0 = half * Sq
        in_maps.append(
            {
                "x1": x1[b],
                "x2h": np.ascontiguousarray(x2[b, q0:q0 + Sq]),
                "biasT": np.ascontiguousarray(
                    (attn_bias[b, :, q0:q0 + Sq, :].transpose(0, 2, 1)
                     / SCALE).astype(np_e4)
                ),
                **shared,
            }
        )
    return in_maps


def _assemble(results: list[dict]) -> np.ndarray:
    out = np.empty((B, S, H), np.float32)
    for c in range(8):
        b, half = c // 2, c % 2
        out[b, half * Sq:(half + 1) * Sq] = results[c]["out"]
    return out


def run(inputs: dict, **run_kwargs):
    from concourse.bass_utils import run_bass_kernel_spmd

    ln_affine, with_biases = _detect_fast_flags(inputs)
    nc = _get_program(ln_affine=ln_affine, with_biases=with_biases)
    in_maps = _make_in_maps(inputs)
    res = run_bass_kernel_spmd(nc, in_maps, core_ids=list(range(8)), **run_kwargs)
    return _assemble(res.results), res


def kernel(**inputs) -> np.ndarray:
    out, _ = run(inputs)
    return out

